# revision 58
# baseline (speedup 1.0000x reference)
"""Single-head causal self-attention on 8 trn2 NeuronCores.

Problem: x[4,4096,1024], Wq/Wk/Wv[1024,128]+biases -> causal attention out
[4,4096,128], fp32.

Sharding: core c = (b = c//2, j = c%2). Core (b, j) handles batch b and the
K/V column 128-blocks of parity j (alternating blocks balance the causal
triangle). It computes, for ALL 4096 query rows, the *unnormalized* partial
attention over its own columns:
    O_un^T[h, s] = sum_{t in cols_j, t<=s} exp(q_s.k_t * scale) * v_t[h]
    l[s]         = sum_{t in cols_j, t<=s} exp(q_s.k_t * scale)
Host combines:  O[s] = (O_un0[s] + O_un1[s]) / (l0[s] + l1[s]) + bv.
No per-core max subtraction is needed: scores are ~N(0,1) (bounded ~6), so
exp never overflows; masked entries get multiplied by a 0/1 mask after exp.

Bias algebra: softmax((q+bq).(k+bk)) == softmax((q+bq).k) because the
(q+bq).bk term is constant in t and cancels in the combined normalization
(both cores use the same biased-Q/unbiased-K convention). The V bias moves
to the host: O = sum w (v+bv) / sum w = sum w v / sum w + bv. So the device
applies only the Q bias.

SPMD uniformity: the same Bass program runs on all 8 cores. Parity enters
only through data: for j=1 the host swaps adjacent 128-row blocks of x
(involution), so "even position blocks" on the device are the core's own
columns; the causal diagonal mask ([128,768] trimmed layout, R-independent
by algebra) is passed as a per-core input. Output comes back in position space and the
host un-swaps.

Precision: all matmul operands are bf16 (x^T, W, Q^T, K^T, V, P); PSUM
accumulation is f32; exp input (scores) is f32. Measured end-to-end rel
err ~4e-3 vs the f32 reference (tolerance 2e-2).

Device pipeline per core (per 512-row superstep i2, emission order tuned so
every engine dependency hides under other PE work):
  A_q: Q^T projection (W stationary, 8 E-chunks into PSUM; DVE adds bias,
    writes bf16). Pairs (i2, p<=i2-3) interleave here -- they need only
    qt(i2) and OLD kt/v blocks.
  A_k: K^T projection for the 2 own t-blocks (DVE copy -> bf16); pair i2-2
    runs next so the kt-copy latency hides under its S/AV work.
  A_v: V^T projection (ACT copy -> bf16 vt_tmp, PE-transpose via bf16
    identity, DVE copy -> V[t,h]); then the diagonal pair, then pair i2-1.
  Pair task (R, p): S^T[t,s] = K^T.T @ Q^T (PSUM f32, 2 banks); ACT exp
    (scale folded) -> bf16. The first-emitted pair's exp lands directly in
    the l-accumulator; later pairs DVE-add into it. The diagonal pair is
    TRIMMED: block 2R+1 only computes s in [256:512) (rest fully masked),
    its [128,768] exp gets the 0/1 mask (DVE mul).
    AV: O^T += V.T @ P^T (PSUM accum across all pairs of R).
    After the last pair: l[1,512] = ones.T @ acc (2 matmuls PSUM-accumulated;
    at R=7 the final pair bypasses the accumulator and streams into 2 extra
    l matmuls, and the diagonal is pulled forward so the end-of-kernel chain
    runs through mask-free pairs) -> drains (o on ACT for R<5 else DVE; o as
    bf16) -> DMA.
  ~64 zero-matmul PE warmups bridge the head DMA latency (pstate/HAM ramp).
"""

import sys

sys.path.insert(0, "/opt/trn_rl_repo")

import numpy as np

import concourse.bacc as bacc
import concourse.mybir as mybir
import concourse.tile as tile
from concourse import bass_utils
from concourse.masks import make_identity

S, E, H, B = 4096, 1024, 128, 4
NSUP, SUP = 8, 512
SCALE = 1.0 / float(np.sqrt(128.0))
F32 = mybir.dt.float32
BF16 = mybir.dt.bfloat16
ACT_IDENT = mybir.ActivationFunctionType.Identity
ACT_EXP = mybir.ActivationFunctionType.Exp

try:
    import ml_dtypes

    NP_BF16 = ml_dtypes.bfloat16
except ImportError:  # pragma: no cover
    NP_BF16 = None


def build_nc(loop_n=None, warmup=64, xt_pieces=1, ptb=3, ob=2, vb=3, ab=3, xb=3, j1=0, j2=0, j3=0):
    nc = bacc.Bacc("TRN2", debug=False, num_devices=8)
    xt_d = nc.dram_tensor("xt", [E, S], BF16, kind="ExternalInput").ap()
    # weights arrive host-pre-arranged as [p, c*H] so DMA descriptors are
    # 2KB/partition (full line rate) instead of 256B (2x latency penalty)
    wq_d = nc.dram_tensor("wq", [128, 8 * H], BF16, kind="ExternalInput").ap()
    wk_d = nc.dram_tensor("wk", [128, 8 * H], BF16, kind="ExternalInput").ap()
    wv_d = nc.dram_tensor("wv", [128, 8 * H], BF16, kind="ExternalInput").ap()
    bias_d = nc.dram_tensor("bias", [H, 1], F32, kind="ExternalInput").ap()
    # trimmed diagonal mask: cols [0:512] = first block over all s, cols
    # [512:768] = second block over s in [256:512) (no unmasked content below)
    mask_d = nc.dram_tensor("mask", [128, 768], BF16, kind="ExternalInput").ap()
    outT_d = nc.dram_tensor("outT", [H, S], BF16, kind="ExternalOutput").ap()
    l_d = nc.dram_tensor("lsum", [1, S], F32, kind="ExternalOutput").ap()

    with tile.TileContext(nc) as tc:
        with (
            tc.tile_pool(name="persist", bufs=1) as pp,
            tc.tile_pool(name="xts", bufs=xb) as xtp,
            tc.tile_pool(name="vtmp", bufs=vb) as vtp,
            tc.tile_pool(name="pts", bufs=ptb) as ptp,
            tc.tile_pool(name="accs", bufs=ab) as accp,
            tc.tile_pool(name="osb", bufs=ob) as osp,
            tc.tile_pool(name="psproj", bufs=2, space="PSUM") as prp,
            tc.tile_pool(name="psst", bufs=2, space="PSUM") as stp,
            tc.tile_pool(name="pso", bufs=1, space="PSUM") as outp,
            tc.tile_pool(name="psl", bufs=1, space="PSUM") as lp,
        ):
            xt_view = xt_d.rearrange("(c p) s -> p c s", p=128)
            xT_tiles = {}

            def dma_xT_piece(i2, lo, w):
                if i2 not in xT_tiles:
                    xT_tiles[i2] = xtp.tile([128, 8, SUP], BF16, tag="xT", name="xT")
                nc.sync.dma_start(
                    xT_tiles[i2][:, lo : lo + w, :],
                    xt_view[:, lo : lo + w, i2 * SUP : (i2 + 1) * SUP],
                )

            def dma_xT(i2):
                w = 8 // xt_pieces
                for hh in range(xt_pieces):
                    dma_xT_piece(i2, w * hh, w)

            # Head DMA order tuned so S(0,0)'s gating inputs (wq, bias, xT0,
            # wk) transfer first on the serialized DMA bus; wv/mask can land
            # ~1.5us later (AV(0,0) waits on exp anyway); xT1 right after.
            w_r = {}
            wq_s = pp.tile([128, 8, H], BF16, name="wr_q")
            wq_view = wq_d.rearrange("p (c h) -> p c h", c=8)
            nc.sync.dma_start(wq_s[:, 0:1, :], wq_view[:, 0:1, :])
            w_r["q"] = wq_s
            bias_q = pp.tile([128, 1], F32)
            nc.sync.dma_start(bias_q, bias_d)
            dma_xT_piece(0, 0, 4)
            nc.sync.dma_start(wq_s[:, 1:8, :], wq_view[:, 1:8, :])
            dma_xT_piece(0, 4, 4)
            wr_k = pp.tile([128, 8, H], BF16, name="wr_k")
            nc.sync.dma_start(wr_k, wk_d.rearrange("p (c h) -> p c h", c=8))
            w_r["k"] = wr_k
            dma_xT_piece(1, 0, 4)
            wr_v = pp.tile([128, 8, H], BF16, name="wr_v")
            nc.sync.dma_start(wr_v, wv_d.rearrange("p (c h) -> p c h", c=8))
            w_r["v"] = wr_v
            dma_xT_piece(1, 4, 4)
            mask_s = pp.tile([128, 768], BF16)
            nc.sync.dma_start(mask_s, mask_d)

            ident = pp.tile([128, 128], BF16)
            make_identity(nc, ident)
            ones_col = pp.tile([128, 1], BF16)
            nc.vector.memset(ones_col, 1.0)

            # PE warm-up: dummy matmuls with no DMA deps bridge the head DMA
            # latency so the pstate ramp completes before (and PE never idles
            # ahead of) the first projection matmul. Source tile comes from a
            # fast DVE memset so warmups start within ~0.3us.
            if warmup:
                warm_src = pp.tile([128, 128], BF16, name="warm_src")
                nc.vector.memset(warm_src, 0.0)
                warm_ps = prp.tile([128, 128], F32, tag="proj", name="warm_ps")
                for _ in range(warmup):
                    nc.tensor.matmul(warm_ps, warm_src, warm_src, start=True, stop=True)

            qt_all = pp.tile([128, S], BF16)
            kt_all = pp.tile([128, 16, 128], BF16)
            v_all = pp.tile([128, 16, 128], BF16)

            o_ps = {}
            l_ps = {}
            acc_of = {}
            pt_of = {}
            # emission-order first/last flags per (R, p), filled by emit_body
            seq_of = {}

            def emit_junk(n):
                if not n:
                    return
                # PE keep-warm fillers for known head DMA/drain-latency gaps:
                # they run immediately (no DMA deps) so the pstate ramp does
                # not reset, and the post-gap matmuls issue at full rate.
                # Target the l-sum PSUM bank, which is idle until R=0 ends.
                jt = lp.tile([128, 128], F32, tag="l", name="junk_ps")
                for _ in range(n):
                    nc.tensor.matmul(jt, warm_src, warm_src, start=True, stop=True)

            qt_ps_of = {}

            def emit_A_q_mms(i2, lo, hi):
                if lo == 0:
                    if i2 + 2 < NSUP:
                        dma_xT(i2 + 2)
                    qt_ps_of[i2] = prp.tile([128, SUP], F32, tag="proj", name="qt_ps")
                xT = xT_tiles[i2]
                for c in range(lo, hi):
                    nc.tensor.matmul(
                        qt_ps_of[i2],
                        w_r["q"][:, c, :],
                        xT[:, c, :],
                        start=(c == 0),
                        stop=(c == 7),
                    )

            def emit_A_q_bias(i2):
                nc.vector.tensor_scalar_add(
                    qt_all[:, i2 * SUP : (i2 + 1) * SUP], qt_ps_of.pop(i2), bias_q
                )

            def emit_A_q(i2):
                emit_A_q_mms(i2, 0, 8)
                emit_A_q_bias(i2)

            def emit_A_k(i2):
                xT = xT_tiles[i2]
                xT4 = xT.rearrange("p c (t w) -> p c t w", t=4)
                kt_ps = prp.tile([128, 256], F32, tag="proj", name="kt_ps")
                for c in range(8):
                    nc.tensor.matmul(
                        kt_ps,
                        w_r["k"][:, c, :],
                        xT4[:, c, 0::2, :],
                        start=(c == 0),
                        stop=(c == 7),
                    )
                nc.vector.tensor_copy(
                    kt_all[:, 2 * i2 : 2 * i2 + 2, :],
                    kt_ps.rearrange("p (t w) -> p t w", t=2),
                )

            vt_tmps = {}

            def emit_A_v_mm(i2):
                xT = xT_tiles.pop(i2)
                xT4 = xT.rearrange("p c (t w) -> p c t w", t=4)
                vt_ps = prp.tile([128, 256], F32, tag="proj", name="vt_ps")
                for c in range(8):
                    nc.tensor.matmul(
                        vt_ps,
                        w_r["v"][:, c, :],
                        xT4[:, c, 0::2, :],
                        start=(c == 0),
                        stop=(c == 7),
                    )
                vt_tmp = vtp.tile([128, 256], BF16, tag="vtmp", name="vt_tmp")
                nc.scalar.activation(vt_tmp, vt_ps, ACT_IDENT)
                vt_tmps[i2] = vt_tmp

            def emit_A_v_fin(i2):
                vt_tmp = vt_tmps.pop(i2)
                v_ps2 = prp.tile([128, 256], BF16, tag="proj", name="v_ps2")
                for tt in range(2):
                    nc.tensor.transpose(
                        v_ps2[:, tt * 128 : (tt + 1) * 128],
                        vt_tmp[:, tt * 128 : (tt + 1) * 128],
                        ident,
                    )
                nc.vector.tensor_copy(
                    v_all[:, 2 * i2 : 2 * i2 + 2, :],
                    v_ps2.rearrange("p (t w) -> p t w", t=2),
                )

            def emit_A_v(i2):
                emit_A_v_mm(i2)
                emit_A_v_fin(i2)

            def emit_S(task):
                R, p = task
                first_p, _ = seq_of[task]
                qt_R = qt_all[:, R * SUP : (R + 1) * SUP]
                if p == R:
                    # diagonal pair, trimmed: block 2R over all 512 s-cols,
                    # block 2R+1 only over s in [256:512) (rest is masked)
                    st = stp.tile([128, 768], F32, tag="st", name="st_d")
                    nc.tensor.matmul(
                        st[:, 0:SUP], kt_all[:, 2 * R, :], qt_R, start=True, stop=True
                    )
                    nc.tensor.matmul(
                        st[:, SUP : SUP + 256],
                        kt_all[:, 2 * R + 1, :],
                        qt_all[:, R * SUP + 256 : (R + 1) * SUP],
                        start=True,
                        stop=True,
                    )
                    if first_p:
                        pt = accp.tile([128, 768], BF16, tag="acc", name="acc_d")
                        acc_of[R] = pt
                    else:
                        pt = ptp.tile([128, 768], BF16, tag="pt", name="pt_d")
                    nc.scalar.activation(pt, st, ACT_EXP, scale=SCALE)
                    nc.vector.tensor_mul(pt, pt, mask_s)
                    pt_of[task] = pt
                else:
                    st = stp.tile([128, 2, SUP], F32, tag="st", name="st")
                    for half in range(2):
                        k = 2 * p + half
                        nc.tensor.matmul(
                            st[:, half, :], kt_all[:, k, :], qt_R, start=True, stop=True
                        )
                    if first_p:
                        # first-emitted pair's exp lands straight in the l-acc
                        pt = accp.tile([128, 2, SUP], BF16, tag="acc", name="acc")
                        acc_of[R] = pt
                    else:
                        pt = ptp.tile([128, 2, SUP], BF16, tag="pt", name="pt")
                    nc.scalar.activation(pt, st, ACT_EXP, scale=SCALE)
                    pt_of[task] = pt

            def emit_AV(task):
                R, p = task
                first_p, last_p = seq_of[task]
                if first_p:
                    o_ps[R] = outp.tile([128, SUP], F32, tag="o", name="o_ps")
                pt = pt_of.pop(task)
                diag = p == R
                # skip the final DVE accumulate on the very last pair of the
                # last superblock: its contribution is streamed straight into
                # the l matmuls instead (shorter end-of-kernel chain)
                skip_add = last_p and R == NSUP - 1 and not first_p
                if not first_p and not skip_add:
                    # accumulate into the l-sum tile; emitted here (one task
                    # after emit_S) so it orders AFTER emit_AV(first)'s reads
                    # of acc — the first pair's AV consumes acc as its P.
                    acc = acc_of[R]
                    if diag:
                        accf = acc.rearrange("p a b -> p (a b)")
                        nc.vector.tensor_add(accf[:, 0:SUP], accf[:, 0:SUP], pt[:, 0:SUP])
                        nc.vector.tensor_add(
                            accf[:, SUP + 256 : 2 * SUP],
                            accf[:, SUP + 256 : 2 * SUP],
                            pt[:, SUP : SUP + 256],
                        )
                    else:
                        nc.vector.tensor_add(acc, acc, pt)
                if diag:
                    nc.tensor.matmul(
                        o_ps[R],
                        v_all[:, 2 * R, :],
                        pt[:, 0:SUP],
                        start=first_p,
                        stop=False,
                    )
                    nc.tensor.matmul(
                        o_ps[R][:, 256:SUP],
                        v_all[:, 2 * R + 1, :],
                        pt[:, SUP : SUP + 256],
                        start=False,
                        stop=last_p,
                    )
                else:
                    for half in range(2):
                        k = 2 * p + half
                        nc.tensor.matmul(
                            o_ps[R],
                            v_all[:, k, :],
                            pt[:, half, :],
                            start=(first_p and half == 0),
                            stop=(last_p and half == 1),
                        )
                if last_p:
                    acc = acc_of.pop(R)
                    l_ps[R] = lp.tile([128, SUP], F32, tag="l", name="l_ps")
                    if R == 0:
                        # acc is the trimmed diagonal tile [128, 768]
                        nc.tensor.matmul(
                            l_ps[R][0:1, :], ones_col, acc[:, 0:SUP], start=True, stop=False
                        )
                        nc.tensor.matmul(
                            l_ps[R][0:1, 256:SUP],
                            ones_col,
                            acc[:, SUP : SUP + 256],
                            start=False,
                            stop=True,
                        )
                    else:
                        l_srcs = [acc[:, 0, :], acc[:, 1, :]]
                        if skip_add:
                            l_srcs += [pt[:, 0, :], pt[:, 1, :]]
                        for i, src in enumerate(l_srcs):
                            nc.tensor.matmul(
                                l_ps[R][0:1, :],
                                ones_col,
                                src,
                                start=(i == 0),
                                stop=(i == len(l_srcs) - 1),
                            )
                    o_sb = osp.tile([128, SUP], BF16, tag="o_sb", name="o_sb")
                    # ACT paces the last superblocks (8 exps each); route the
                    # late o drains to DVE which has slack there
                    if R < 5:
                        nc.scalar.activation(o_sb, o_ps[R], ACT_IDENT)
                    else:
                        nc.vector.tensor_copy(o_sb, o_ps[R])
                    nc.sync.dma_start(outT_d[:, R * SUP : (R + 1) * SUP], o_sb)
                    l_sb = osp.tile([1, SUP], F32, tag="l_sb", name="l_sb")
                    nc.vector.tensor_copy(l_sb, l_ps[R][0:1, :])
                    nc.sync.dma_start(l_d[:, R * SUP : (R + 1) * SUP], l_sb)

            pipe = {"prev": None}

            def push_task(task):
                emit_S(task)
                if pipe["prev"] is not None:
                    emit_AV(pipe["prev"])
                pipe["prev"] = task

            def emit_body():
                pipe["prev"] = None
                # supersteps 0 and 1 are DMA-bus-paced; hand-interleave them
                # so each PE-FIFO slot matches its input's arrival order:
                # Q(1) chunks 0-3 (xT1a) slot in before the wv-gated V(0)
                # work, chunks 4-7 (xT1b) after it.
                seq_of[(0, 0)] = (True, True)
                seq_of[(1, 0)] = (True, False)
                seq_of[(1, 1)] = (False, True)
                emit_A_q(0)
                emit_junk(j1)
                emit_A_k(0)
                emit_junk(j2)
                push_task((0, 0))
                emit_A_q_mms(1, 0, 4)
                emit_A_v(0)
                emit_A_q_mms(1, 4, 8)
                emit_A_q_bias(1)
                push_task((1, 0))
                emit_A_k(1)
                emit_A_v_mm(1)
                emit_junk(j3)
                emit_A_v_fin(1)
                push_task((1, 1))
                for i2 in range(2, NSUP):
                    # non-diagonal pairs p<=i2-3 need only qt(i2) and OLD
                    # kt/v blocks, so they interleave with this superstep's
                    # projections; pair i2-2 sits between A_k and A_v so the
                    # fresh kt DVE-copy latency hides under its S/AV work;
                    # the diagonal goes after A_v (its S covers the
                    # vt-copy->transpose latency) and pair i2-1 last keeps
                    # the 1-deep S->AV pipeline around the diagonal.
                    if i2 == NSUP - 1:
                        # last superblock: pull the diagonal (and its DVE
                        # mask-mul) forward so the end-of-kernel chain runs
                        # through mask-free pairs only
                        before = list(range(i2 - 3))
                        after_k = [i2 - 3, i2]
                        after_v = [i2 - 2, i2 - 1]
                    else:
                        before = list(range(i2 - 2))
                        after_k = [i2 - 2]
                        after_v = [i2, i2 - 1]
                    order = before + after_k + after_v
                    for _i, _p in enumerate(order):
                        seq_of[(i2, _p)] = (_i == 0, _i == len(order) - 1)
                    emit_A_q(i2)
                    for p in before:
                        push_task((i2, p))
                    emit_A_k(i2)
                    for p in after_k:
                        push_task((i2, p))
                    emit_A_v(i2)
                    for p in after_v:
                        push_task((i2, p))
                emit_AV(pipe["prev"])

            if loop_n is None:
                emit_body()
            else:
                with tc.For_i(0, loop_n, 1):
                    emit_body()

    nc.compile()
    return nc


def _perm1():
    idx = np.arange(S)
    return (idx // 128 ^ 1) * 128 + idx % 128


def _mask_for(j):
    ti = np.arange(128)[:, None, None]
    m = np.arange(2)[None, :, None]
    si = np.arange(SUP)[None, None, :]
    orig_s = 128 * ((si // 128) ^ j) + si % 128
    vis = (orig_s >= 128 * (2 * m + j) + ti).astype(np.float32)
    # trimmed layout [128, 768]: block 0 over all 512 s, block 1 only over
    # s in [256:512) (the rest of block 1 has no unmasked content)
    return np.concatenate([vis[:, 0, :], vis[:, 1, 256:]], axis=1).astype(NP_BF16)


_CACHE = {}


def kernel(x, Wq, bq, Wk, bk, Wv, bv):
    if "nc" not in _CACHE:
        _CACHE["nc"] = build_nc()
    nc = _CACHE["nc"]

    def _w_arrange(W):
        # [E, H] -> [128, 8*H] with [p, c*H+h] = W[c*128+p, h]
        W = np.asarray(W, dtype=np.float32).astype(NP_BF16)
        return np.ascontiguousarray(
            W.reshape(8, 128, H).transpose(1, 0, 2).reshape(128, 8 * H)
        )

    x = np.asarray(x, dtype=np.float32)
    Wq = _w_arrange(Wq)
    Wk = _w_arrange(Wk)
    Wv = _w_arrange(Wv)
    bq = np.ascontiguousarray(np.asarray(bq, dtype=np.float32))
    bv = np.asarray(bv, dtype=np.float32)

    perm = _perm1()
    masks = {j: _mask_for(j) for j in (0, 1)}
    # x^T per batch (bf16), and the column-block-swapped variant for parity-1
    xT = {}
    for b in range(B):
        t = np.ascontiguousarray(x[b].T.astype(NP_BF16))  # [E, S]
        xT[(b, 0)] = t
        xT[(b, 1)] = np.ascontiguousarray(
            t.reshape(E, S // 128, 128)[:, (np.arange(S // 128) ^ 1), :].reshape(E, S)
        )

    in_maps = []
    for c in range(8):
        b, j = divmod(c, 2)
        in_maps.append(
            {
                "xt": xT[(b, j)],
                "wq": Wq,
                "wk": Wk,
                "wv": Wv,
                "bias": np.ascontiguousarray(bq.reshape(H, 1)),
                "mask": masks[j],
            }
        )

    res = bass_utils.run_bass_kernel_spmd(nc, in_maps, core_ids=list(range(8)))

    out = np.empty((B, S, H), np.float32)
    for b in range(B):
        oT0 = res.results[2 * b]["outT"].astype(np.float32)
        l0 = res.results[2 * b]["lsum"][0]
        oT1 = res.results[2 * b + 1]["outT"].astype(np.float32)[:, perm]
        l1 = res.results[2 * b + 1]["lsum"][0][perm]
        out[b] = ((oT0 + oT1) / (l0 + l1)[None, :]).T + bv[None, :]
    return out
